# revision 1
# baseline (speedup 1.0000x reference)
"""BiATT kernel for 8 Trainium2 NeuronCores.

The reference module's bilinear-attention branch is dead code: the
"attention" weights are softmax(axis=1) over [N, 1] tensors, which is
exactly 1.0 for every row.  Hence

    cf_final = atoms_vector @ (Wcc[0:D] + Wcc[D:2D] + Wcc[2D:3D] + Wcc[3D:4D]) + bcc
    pf_final = amino_vector @ (Wcp[0:D] + Wcp[D:2D] + Wcp[2D:3D] + Wcp[3D:4D]) + bcp

bit-for-bit up to fp32 rounding.  The device kernel therefore computes two
[768, 512] @ [512, 512] matmuls per core (rows sharded 8 ways, folded
weights replicated).

Layout: the device computes cf.T / pf.T ([512 features, 768 rows]) with the
feature dim on SBUF partitions so the bias is a per-partition scalar add on
the DVE; the host pre-transposes the row shards and transposes the outputs
back when gathering.
"""

import os

import numpy as np

import concourse.bacc as bacc
import concourse.bass as bass
import concourse.mybir as mybir
import concourse.tile as tile
from concourse.bass_utils import run_bass_kernel_spmd

N_CORES = 8
D = 512          # feature dim
N_ROWS = 6144    # rows of atoms_vector / amino_vector
SHARD = N_ROWS // N_CORES   # 768 rows per core
P = 128          # SBUF partitions
KC = D // P      # 4 contraction chunks
MB = D // P      # 4 output-feature blocks
NT = 2           # row-slice tiles per shard
NS = SHARD // NT  # 384 rows per matmul (one PSUM bank each)

_F32 = mybir.dt.float32
_PROGRAM_CACHE = {}

_LAST_EXEC_NS = None


def _build_program(mm_dtype):
    """SPMD program: one core's shard.  Inputs are laid out by the host so
    every DMA is a contiguous K-chunk:
      xT / yT : [KC, 128, SHARD]  (pre-transposed row shard, K-chunked)
      wcc/wcp : [KC, 128, D]      (folded weight, K-chunked rows)
      bcc/bcp : [MB, 128, 1]      (bias per output feature)
    Outputs cfT / pfT : [D, SHARD].
    """
    nc = bacc.Bacc(
        "TRN2",
        target_bir_lowering=False,
        debug=False,
        num_devices=N_CORES,
    )

    xT = nc.dram_tensor("xT", [KC, P, SHARD], mm_dtype, kind="ExternalInput").ap()
    yT = nc.dram_tensor("yT", [KC, P, SHARD], mm_dtype, kind="ExternalInput").ap()
    wcc = nc.dram_tensor("wcc", [KC, P, D], mm_dtype, kind="ExternalInput").ap()
    wcp = nc.dram_tensor("wcp", [KC, P, D], mm_dtype, kind="ExternalInput").ap()
    bcc = nc.dram_tensor("bcc", [MB, P, 1], _F32, kind="ExternalInput").ap()
    bcp = nc.dram_tensor("bcp", [MB, P, 1], _F32, kind="ExternalInput").ap()
    cfT = nc.dram_tensor("cfT", [D, SHARD], _F32, kind="ExternalOutput").ap()
    pfT = nc.dram_tensor("pfT", [D, SHARD], _F32, kind="ExternalOutput").ap()

    with tile.TileContext(nc) as tc:
        with (
            tc.tile_pool(name="ins", bufs=1) as ins,
            tc.tile_pool(name="psum", bufs=8, space=bass.MemorySpace.PSUM) as psum,
            tc.tile_pool(name="outs", bufs=8) as outs,
        ):
            # Stage inputs.  Emit in consumption order (cf stream first) so
            # the PE can start as soon as the first K-chunk pair lands.
            w1_t, x_t, w2_t, y_t = [], [], [], []
            for k in range(KC):
                wt = ins.tile([P, D], mm_dtype, tag=f"wcc{k}")
                nc.sync.dma_start(wt[:], wcc[k])
                w1_t.append(wt)
                xt_ = ins.tile([P, SHARD], mm_dtype, tag=f"x{k}")
                nc.sync.dma_start(xt_[:], xT[k])
                x_t.append(xt_)
            b1_t = []
            for m in range(MB):
                bt = ins.tile([P, 1], _F32, tag=f"bcc{m}")
                nc.sync.dma_start(bt[:], bcc[m])
                b1_t.append(bt)
            for k in range(KC):
                wt = ins.tile([P, D], mm_dtype, tag=f"wcp{k}")
                nc.sync.dma_start(wt[:], wcp[k])
                w2_t.append(wt)
                yt_ = ins.tile([P, SHARD], mm_dtype, tag=f"y{k}")
                nc.sync.dma_start(yt_[:], yT[k])
                y_t.append(yt_)
            b2_t = []
            for m in range(MB):
                bt = ins.tile([P, 1], _F32, tag=f"bcp{m}")
                nc.sync.dma_start(bt[:], bcp[m])
                b2_t.append(bt)

            # outT[m*128:(m+1)*128, n*NS:(n+1)*NS] =
            #     sum_k w[k][:, mblock].T @ act[k][:, nslice]  + bias[m]
            for w_t, a_t, b_t, out_d in (
                (w1_t, x_t, b1_t, cfT),
                (w2_t, y_t, b2_t, pfT),
            ):
                for m in range(MB):
                    for n in range(NT):
                        ps = psum.tile([P, NS], _F32, tag="ps")
                        for k in range(KC):
                            nc.tensor.matmul(
                                ps[:],
                                w_t[k][:, m * P:(m + 1) * P],
                                a_t[k][:, n * NS:(n + 1) * NS],
                                start=(k == 0),
                                stop=(k == KC - 1),
                            )
                        ot = outs.tile([P, NS], _F32, tag="ot")
                        nc.vector.tensor_scalar_add(ot[:], ps[:], b_t[m][:, 0:1])
                        nc.sync.dma_start(
                            out_d[m * P:(m + 1) * P, n * NS:(n + 1) * NS], ot[:]
                        )

    nc.compile()
    return nc


def _get_program(mm_dtype):
    key = str(mm_dtype)
    if key not in _PROGRAM_CACHE:
        _PROGRAM_CACHE[key] = _build_program(mm_dtype)
    return _PROGRAM_CACHE[key]


def kernel(**inputs):
    global _LAST_EXEC_NS

    atoms = np.ascontiguousarray(np.asarray(inputs["atoms_vector"], dtype=np.float32))
    amino = np.ascontiguousarray(np.asarray(inputs["amino_vector"], dtype=np.float32))
    Wcc = np.asarray(inputs["Wcc"], dtype=np.float32)
    Wcp = np.asarray(inputs["Wcp"], dtype=np.float32)
    bcc = np.asarray(inputs["bcc"], dtype=np.float32)
    bcp = np.asarray(inputs["bcp"], dtype=np.float32)

    # Fold the four weight blocks (concat([v]*4, 1) @ W == v @ sum-of-blocks).
    wcc_f = np.ascontiguousarray(Wcc.reshape(4, D, D).sum(axis=0)).reshape(KC, P, D)
    wcp_f = np.ascontiguousarray(Wcp.reshape(4, D, D).sum(axis=0)).reshape(KC, P, D)
    bcc_r = np.ascontiguousarray(bcc.reshape(MB, P, 1))
    bcp_r = np.ascontiguousarray(bcp.reshape(MB, P, 1))

    mm_dtype = _F32
    nc = _get_program(mm_dtype)

    in_maps = []
    for c in range(N_CORES):
        sl = slice(c * SHARD, (c + 1) * SHARD)
        in_maps.append({
            "xT": np.ascontiguousarray(atoms[sl].T).reshape(KC, P, SHARD),
            "yT": np.ascontiguousarray(amino[sl].T).reshape(KC, P, SHARD),
            "wcc": wcc_f,
            "wcp": wcp_f,
            "bcc": bcc_r,
            "bcp": bcp_r,
        })

    trace = bool(os.environ.get("BIATT_TRACE"))
    res = run_bass_kernel_spmd(nc, in_maps, list(range(N_CORES)), trace=trace)
    _LAST_EXEC_NS = res.exec_time_ns

    cf = np.concatenate([res.results[c]["cfT"].T for c in range(N_CORES)], axis=0)
    pf = np.concatenate([res.results[c]["pfT"].T for c in range(N_CORES)], axis=0)
    return np.ascontiguousarray(cf), np.ascontiguousarray(pf)


# revision 2
# speedup vs baseline: 1.3992x; 1.3992x over previous
"""BiATT kernel for 8 Trainium2 NeuronCores.

The reference module's bilinear-attention branch is dead code: the
"attention" weights are softmax(axis=1) over [N, 1] tensors, which is
exactly 1.0 for every row.  Hence

    cf_final = atoms_vector @ (Wcc[0:D] + Wcc[D:2D] + Wcc[2D:3D] + Wcc[3D:4D]) + bcc
    pf_final = amino_vector @ (Wcp[0:D] + Wcp[D:2D] + Wcp[2D:3D] + Wcp[3D:4D]) + bcp

bit-for-bit up to fp32 rounding.  The device kernel therefore computes two
[768, 512] @ [512, 512] matmuls per core (rows sharded 8 ways, folded
weights replicated).

Layout: the device computes cf.T / pf.T ([512 features, 768 rows]) with the
feature dim on SBUF partitions so the bias is a per-partition scalar add on
the DVE; the host pre-transposes the row shards and transposes the outputs
back when gathering.
"""

import os

import numpy as np

import concourse.bacc as bacc
import concourse.bass as bass
import concourse.mybir as mybir
import concourse.tile as tile
from concourse.bass_utils import run_bass_kernel_spmd

N_CORES = 8
D = 512          # feature dim
N_ROWS = 6144    # rows of atoms_vector / amino_vector
SHARD = N_ROWS // N_CORES   # 768 rows per core
P = 128          # SBUF partitions
KC = D // P      # 4 contraction chunks
MB = D // P      # 4 output-feature blocks
NT = 2           # row-slice tiles per shard
NS = SHARD // NT  # 384 rows per matmul (one PSUM bank each)

_F32 = mybir.dt.float32
_PROGRAM_CACHE = {}

_LAST_EXEC_NS = None


def _build_program(mm_dtype):
    """SPMD program: one core's shard.  Inputs are laid out by the host so
    every DMA is a contiguous K-chunk:
      xT / yT : [KC, 128, SHARD]  (pre-transposed row shard, K-chunked)
      wcc/wcp : [KC, 128, D]      (folded weight, K-chunked rows)
      bcc/bcp : [MB, 128, 1]      (bias per output feature)
    Outputs cfT / pfT : [D, SHARD].
    """
    nc = bacc.Bacc(
        "TRN2",
        target_bir_lowering=False,
        debug=False,
        num_devices=N_CORES,
    )

    xT = nc.dram_tensor("xT", [KC, P, SHARD], mm_dtype, kind="ExternalInput").ap()
    yT = nc.dram_tensor("yT", [KC, P, SHARD], mm_dtype, kind="ExternalInput").ap()
    wcc = nc.dram_tensor("wcc", [KC, P, D], mm_dtype, kind="ExternalInput").ap()
    wcp = nc.dram_tensor("wcp", [KC, P, D], mm_dtype, kind="ExternalInput").ap()
    bcc = nc.dram_tensor("bcc", [MB, P, 1], _F32, kind="ExternalInput").ap()
    bcp = nc.dram_tensor("bcp", [MB, P, 1], _F32, kind="ExternalInput").ap()
    cfT = nc.dram_tensor("cfT", [D, SHARD], _F32, kind="ExternalOutput").ap()
    pfT = nc.dram_tensor("pfT", [D, SHARD], _F32, kind="ExternalOutput").ap()

    with tile.TileContext(nc) as tc:
        with (
            tc.tile_pool(name="ins", bufs=1) as ins,
            tc.tile_pool(name="psum", bufs=8, space=bass.MemorySpace.PSUM) as psum,
            tc.tile_pool(name="outs", bufs=8) as outs,
        ):
            # Stage inputs.  Emit in consumption order (cf stream first) so
            # the PE can start as soon as the first K-chunk pair lands.
            w1_t, x_t, w2_t, y_t = [], [], [], []
            for k in range(KC):
                wt = ins.tile([P, D], mm_dtype, tag=f"wcc{k}")
                nc.sync.dma_start(wt[:], wcc[k])
                w1_t.append(wt)
                xt_ = ins.tile([P, SHARD], mm_dtype, tag=f"x{k}")
                nc.sync.dma_start(xt_[:], xT[k])
                x_t.append(xt_)
            b1_t = []
            for m in range(MB):
                bt = ins.tile([P, 1], _F32, tag=f"bcc{m}")
                nc.sync.dma_start(bt[:], bcc[m])
                b1_t.append(bt)
            for k in range(KC):
                wt = ins.tile([P, D], mm_dtype, tag=f"wcp{k}")
                nc.sync.dma_start(wt[:], wcp[k])
                w2_t.append(wt)
                yt_ = ins.tile([P, SHARD], mm_dtype, tag=f"y{k}")
                nc.sync.dma_start(yt_[:], yT[k])
                y_t.append(yt_)
            b2_t = []
            for m in range(MB):
                bt = ins.tile([P, 1], _F32, tag=f"bcp{m}")
                nc.sync.dma_start(bt[:], bcp[m])
                b2_t.append(bt)

            # outT[m*128:(m+1)*128, n*NS:(n+1)*NS] =
            #     sum_k w[k][:, mblock].T @ act[k][:, nslice]  + bias[m]
            for w_t, a_t, b_t, out_d in (
                (w1_t, x_t, b1_t, cfT),
                (w2_t, y_t, b2_t, pfT),
            ):
                for m in range(MB):
                    for n in range(NT):
                        ps = psum.tile([P, NS], _F32, tag="ps")
                        for k in range(KC):
                            nc.tensor.matmul(
                                ps[:],
                                w_t[k][:, m * P:(m + 1) * P],
                                a_t[k][:, n * NS:(n + 1) * NS],
                                start=(k == 0),
                                stop=(k == KC - 1),
                            )
                        ot = outs.tile([P, NS], _F32, tag="ot")
                        nc.vector.tensor_scalar_add(ot[:], ps[:], b_t[m][:, 0:1])
                        nc.sync.dma_start(
                            out_d[m * P:(m + 1) * P, n * NS:(n + 1) * NS], ot[:]
                        )

    nc.compile()
    return nc


def _get_program(mm_dtype):
    key = str(mm_dtype)
    if key not in _PROGRAM_CACHE:
        _PROGRAM_CACHE[key] = _build_program(mm_dtype)
    return _PROGRAM_CACHE[key]


def kernel(**inputs):
    global _LAST_EXEC_NS

    atoms = np.ascontiguousarray(np.asarray(inputs["atoms_vector"], dtype=np.float32))
    amino = np.ascontiguousarray(np.asarray(inputs["amino_vector"], dtype=np.float32))
    Wcc = np.asarray(inputs["Wcc"], dtype=np.float32)
    Wcp = np.asarray(inputs["Wcp"], dtype=np.float32)
    bcc = np.asarray(inputs["bcc"], dtype=np.float32)
    bcp = np.asarray(inputs["bcp"], dtype=np.float32)

    # Fold the four weight blocks (concat([v]*4, 1) @ W == v @ sum-of-blocks).
    wcc_f = np.ascontiguousarray(Wcc.reshape(4, D, D).sum(axis=0)).reshape(KC, P, D)
    wcp_f = np.ascontiguousarray(Wcp.reshape(4, D, D).sum(axis=0)).reshape(KC, P, D)
    bcc_r = np.ascontiguousarray(bcc.reshape(MB, P, 1))
    bcp_r = np.ascontiguousarray(bcp.reshape(MB, P, 1))

    mm_dtype = (
        mybir.dt.float32r
        if os.environ.get("BIATT_MM", "f32r") == "f32r"
        else _F32
    )
    nc = _get_program(mm_dtype)

    in_maps = []
    for c in range(N_CORES):
        sl = slice(c * SHARD, (c + 1) * SHARD)
        in_maps.append({
            "xT": np.ascontiguousarray(atoms[sl].T).reshape(KC, P, SHARD),
            "yT": np.ascontiguousarray(amino[sl].T).reshape(KC, P, SHARD),
            "wcc": wcc_f,
            "wcp": wcp_f,
            "bcc": bcc_r,
            "bcp": bcp_r,
        })

    trace = bool(os.environ.get("BIATT_TRACE"))
    res = run_bass_kernel_spmd(nc, in_maps, list(range(N_CORES)), trace=trace)
    _LAST_EXEC_NS = res.exec_time_ns

    cf = np.concatenate([res.results[c]["cfT"].T for c in range(N_CORES)], axis=0)
    pf = np.concatenate([res.results[c]["pfT"].T for c in range(N_CORES)], axis=0)
    return np.ascontiguousarray(cf), np.ascontiguousarray(pf)
